# revision 9
# baseline (speedup 1.0000x reference)
"""Trainium2 Bass kernel for nn_LBLHighwayBiLm.

Model (hardcoded): L=2 layers x {fw,bw} directions. Per layer+direction:
  5-tap windowed sum along seq (with learned pad vectors), then a 2-deep
  AllenNLP Highway stack (H=1024 -> 2H proj, sigmoid gate + relu), residual
  from layer input for l>0. Output: [L, B, S, 2H] = concat(f, bw).

Strategy: data-parallel over batch across 8 NeuronCores (B=32 -> 4/core).
On-chip layout is feature-major [H(partitions), tokens(free)]:
  - window sums run on DVE as 5 scalar_tensor_tensor taps over a padded
    staging tile, accumulating straight into an fp32r tile,
  - highway projections run on PE as [128,128]x[128,512] fp32r matmuls
    accumulating over H in PSUM (weights are host-pre-rounded to fp32r,
    which is fp32 RNE-rounded to 11 mantissa bits, so they DMA directly
    into fp32r tiles),
  - bias+sigmoid / bias+relu fuse into single ACT ops reading PSUM,
  - the gate combine g*x + (1-g)*relu runs on DVE (3 ops).
Layer-0 outputs round-trip through a DRAM scratch pool for layer 1's
window sum + residual. Host pre-transposes inputs to feature-major and
re-assembles the [L, B, S, 2H] output from per-core [L, 2, H, B/8, S].
"""

import sys

for _p in ("/opt/trn_rl_repo", "/opt/pypackages"):
    if _p not in sys.path:
        sys.path.insert(0, _p)

import numpy as np

import concourse.bass as bass
import concourse.tile as tile
from concourse import mybir, bacc
from concourse import bass_utils

# Problem constants (hardcoded per contract)
L = 2
NH = 2
WIDTH = 4
H = 1024
B, S = 32, 512
CORES = 8
BL = B // CORES          # batch per core
KT = H // 128            # 8 contraction tiles
NT = 2 * H // 128        # 16 output feature tiles
PSEQ = S + 2 * WIDTH     # padded seq length 520

FP32 = mybir.dt.float32
FP32R = mybir.dt.float32r
AF = mybir.ActivationFunctionType
OP = mybir.AluOpType


def rne_round_fp32(x: np.ndarray, mbits: int = 11) -> np.ndarray:
    """Round fp32 to `mbits` explicit mantissa bits (RNE) — the fp32r format."""
    u = np.ascontiguousarray(x, dtype=np.float32).view(np.uint32).astype(np.uint64)
    shift = 23 - mbits
    bias = ((u >> shift) & 1) + ((1 << (shift - 1)) - 1)
    u = (u + bias) & ~np.uint64((1 << shift) - 1)
    return (u & 0xFFFFFFFF).astype(np.uint32).view(np.float32).reshape(x.shape)


def build_nc(loop_n: int = 1):
    """Build the per-core SPMD Bass program. Every core runs the same NEFF
    on its own batch shard (inputs differ, weights replicated).
    loop_n > 1 wraps the body in a hardware loop (timing amplification only)."""
    nc = bacc.Bacc("TRN2", target_bir_lowering=False, debug=False)

    x_t = nc.dram_tensor("x_t", [H, BL, S], FP32, kind="ExternalInput").ap()
    fwpad = nc.dram_tensor("fwpad", [L, H, WIDTH], FP32, kind="ExternalInput").ap()
    bwpad = nc.dram_tensor("bwpad", [L, H, WIDTH], FP32, kind="ExternalInput").ap()
    fw_w = nc.dram_tensor("fw_w", [L, WIDTH + 1], FP32, kind="ExternalInput").ap()
    bw_w = nc.dram_tensor("bw_w", [L, WIDTH + 1], FP32, kind="ExternalInput").ap()
    fw_W = nc.dram_tensor("fw_W", [L, NH, H, 2 * H], FP32R, kind="ExternalInput").ap()
    bw_W = nc.dram_tensor("bw_W", [L, NH, H, 2 * H], FP32R, kind="ExternalInput").ap()
    fw_b = nc.dram_tensor("fw_b", [L, NH, 2 * H], FP32, kind="ExternalInput").ap()
    bw_b = nc.dram_tensor("bw_b", [L, NH, 2 * H], FP32, kind="ExternalInput").ap()
    out = nc.dram_tensor("out", [L, 2, H, BL, S], FP32, kind="ExternalOutput").ap()

    with tile.TileContext(nc) as tc:
        if loop_n == 1:
            _emit(tc, nc, x_t, fwpad, bwpad, fw_w, bw_w, fw_W, bw_W, fw_b, bw_b, out)
        else:
            with tc.For_i(0, loop_n, 1):
                _emit(tc, nc, x_t, fwpad, bwpad, fw_w, bw_w, fw_W, bw_W,
                      fw_b, bw_b, out)
    nc.compile()
    return nc


def _emit(tc, nc, x_t, fwpad, bwpad, fw_w, bw_w, fw_W, bw_W, fw_b, bw_b, out):
    from contextlib import ExitStack
    ctx = ExitStack()
    ws_pool = ctx.enter_context(tc.tile_pool(name="ws", bufs=1))
    x0_pool = ctx.enter_context(tc.tile_pool(name="x0", bufs=1))
    stage_pool = ctx.enter_context(tc.tile_pool(name="stage", bufs=2))
    w_pool = ctx.enter_context(tc.tile_pool(name="wts", bufs=6))
    psum = ctx.enter_context(tc.tile_pool(name="psum", bufs=8, space="PSUM"))
    ract = ctx.enter_context(tc.tile_pool(name="ract", bufs=2))
    sact = ctx.enter_context(tc.tile_pool(name="sact", bufs=2))
    dtmp = ctx.enter_context(tc.tile_pool(name="dtmp", bufs=2))
    ostage = ctx.enter_context(tc.tile_pool(name="ostage", bufs=4))
    resp = ctx.enter_context(tc.tile_pool(name="resp", bufs=4))
    consts = ctx.enter_context(tc.tile_pool(name="consts", bufs=2))
    dram = ctx.enter_context(tc.tile_pool(name="dram", bufs=1, space="DRAM"))

    # layer-0 outputs (pre-concat) kept in DRAM scratch for layer-1 reads
    scr = [dram.tile([H, BL, S], FP32, tag=f"scr{d}", name=f"scr{d}")
           for d in range(2)]

    W_by_dir = (fw_W, bw_W)
    b_by_dir = (fw_b, bw_b)
    w_by_dir = (fw_w, bw_w)

    for l in range(L):
        for d in range(2):  # 0=fw, 1=bw
            # --- window-sum scalars, broadcast to [128, 5]
            wsrc = w_by_dir[d]
            wt = consts.tile([128, WIDTH + 1], FP32, tag="wt")
            nc.sync.dma_start(
                out=wt,
                in_=bass.AP(tensor=wsrc.tensor, offset=l * (WIDTH + 1),
                            ap=[[0, 128], [1, WIDTH + 1]]))

            # --- staging + windowed sum into fp32r ws
            ws = ws_pool.tile([128, KT, BL, S], FP32R, tag="ws")
            offs = 0 if d == 0 else WIDTH
            for kt in range(KT):
                stg = stage_pool.tile([128, BL, PSEQ], FP32, tag="stage")
                if l == 0:
                    body_src = x_t[bass.ts(kt, 128), :, :]
                else:
                    body_src = scr[d][bass.ts(kt, 128), :, :]
                nc.sync.dma_start(out=stg[:, :, WIDTH:WIDTH + S], in_=body_src)
                # NOTE: reference prepends fw_pad and appends bw_pad for BOTH
                # directions' padded sequences.
                fp_ap = fwpad[l, bass.ts(kt, 128), :]
                bp_ap = bwpad[l, bass.ts(kt, 128), :]
                nc.sync.dma_start(
                    out=stg[:, :, 0:WIDTH],
                    in_=bass.AP(tensor=fp_ap.tensor, offset=fp_ap.offset,
                                ap=[fp_ap.ap[0], [0, BL], fp_ap.ap[1]]))
                nc.sync.dma_start(
                    out=stg[:, :, WIDTH + S:PSEQ],
                    in_=bass.AP(tensor=bp_ap.tensor, offset=bp_ap.offset,
                                ap=[bp_ap.ap[0], [0, BL], bp_ap.ap[1]]))
                nc.vector.tensor_scalar(
                    ws[:, kt], stg[:, :, offs:offs + S], wt[:, 0:1], None,
                    op0=OP.mult)
                for k in range(1, WIDTH + 1):
                    nc.vector.scalar_tensor_tensor(
                        ws[:, kt], stg[:, :, offs + k:offs + k + S], wt[:, k:k + 1],
                        ws[:, kt], op0=OP.mult, op1=OP.add)

            # --- highway sublayer 0: ws -> x0 (fp32r)
            x0 = x0_pool.tile([128, KT, BL, S], FP32R, tag="x0")
            _highway(tc, nc, w_pool, psum, ract, sact, dtmp, consts,
                     W_by_dir[d], b_by_dir[d], l, 0, ws, x0=x0)

            # --- highway sublayer 1: x0 -> out (+residual for l>0)
            _highway(tc, nc, w_pool, psum, ract, sact, dtmp, consts,
                     W_by_dir[d], b_by_dir[d], l, 1, x0,
                     nc_out=out, ostage=ostage, resp=resp,
                     scr=scr[d] if l == 0 else None,
                     res_src=scr[d] if l > 0 else None, l_idx=l, d_idx=d)

    ctx.close()


def _highway(tc, nc, w_pool, psum, ract, sact, dtmp, consts,
             W_src, b_src, l, i, x_in, x0=None, nc_out=None, ostage=None,
             resp=None, scr=None, res_src=None, l_idx=None, d_idx=None):
    # bias [2H] -> [128, 16]; column n is features n*128..(n+1)*128
    bt = consts.tile([128, NT], FP32, tag="bt")
    b_ap = b_src[l, i, :]
    nc.sync.dma_start(out=bt, in_=b_ap.rearrange("(n p) -> p n", p=128))

    # weight source view [H, 2H] -> [p, kt, col]
    Wv = W_src[l, i].rearrange("(kt p) c -> p kt c", p=128)

    for jj in range(KT):  # paired feature tiles: nl=jj, gate=jj+8
        Wnl = w_pool.tile([128, KT, 128], FP32R, tag="W")
        nc.sync.dma_start(out=Wnl, in_=Wv[:, :, bass.ts(jj, 128)])
        Wsg = w_pool.tile([128, KT, 128], FP32R, tag="W")
        nc.sync.dma_start(out=Wsg, in_=Wv[:, :, bass.ts(jj + KT, 128)])
        for t in range(BL):
            ps_nl = psum.tile([128, S], FP32, tag="ps")
            ps_sg = psum.tile([128, S], FP32, tag="ps")
            for k in range(KT):
                nc.tensor.matmul(ps_nl, Wnl[:, k], x_in[:, k, t],
                                 start=(k == 0), stop=(k == KT - 1))
            for k in range(KT):
                nc.tensor.matmul(ps_sg, Wsg[:, k], x_in[:, k, t],
                                 start=(k == 0), stop=(k == KT - 1))
            r = ract.tile([128, S], FP32, tag="r")
            nc.scalar.activation(r, ps_nl, AF.Relu, bias=bt[:, jj:jj + 1])
            sg = sact.tile([128, S], FP32, tag="sg")
            nc.scalar.activation(sg, ps_sg, AF.Sigmoid, bias=bt[:, jj + KT:jj + KT + 1])
            dd = dtmp.tile([128, S], FP32, tag="dd")
            nc.vector.tensor_sub(dd, x_in[:, jj, t], r)   # x - relu
            nc.vector.tensor_mul(dd, sg, dd)              # g*(x - relu)
            if x0 is not None:
                # out = g*x + (1-g)*relu = g*(x-relu) + relu
                nc.vector.tensor_add(x0[:, jj, t], dd, r)
            else:
                ot = ostage.tile([128, S], FP32, tag="ot")
                if res_src is None:
                    nc.vector.tensor_add(ot, dd, r)
                else:
                    rs = resp.tile([128, S], FP32, tag="rs")
                    nc.sync.dma_start(out=rs, in_=res_src[bass.ts(jj, 128), t, :])
                    nc.vector.tensor_add(dd, dd, r)
                    nc.vector.tensor_add(ot, dd, rs)      # + layer input (residual)
                nc.sync.dma_start(out=nc_out[l_idx, d_idx, bass.ts(jj, 128), t, :],
                                  in_=ot)
                if scr is not None:
                    nc.sync.dma_start(out=scr[bass.ts(jj, 128), t, :], in_=ot)


def prepare_in_maps(inputs, fw_pad, bw_pad, fw_w, bw_w,
                    fw_hw_W, fw_hw_b, bw_hw_W, bw_hw_b):
    """Shard + lay out host-side. Returns list of 8 per-core input dicts."""
    fw_W_r = rne_round_fp32(fw_hw_W)
    bw_W_r = rne_round_fp32(bw_hw_W)
    fwpad_t = np.ascontiguousarray(np.transpose(
        np.asarray(fw_pad, dtype=np.float32), (0, 2, 1)))
    bwpad_t = np.ascontiguousarray(np.transpose(
        np.asarray(bw_pad, dtype=np.float32), (0, 2, 1)))
    common = {
        "fwpad": fwpad_t, "bwpad": bwpad_t,
        "fw_w": np.ascontiguousarray(fw_w, dtype=np.float32),
        "bw_w": np.ascontiguousarray(bw_w, dtype=np.float32),
        "fw_W": fw_W_r, "bw_W": bw_W_r,
        "fw_b": np.ascontiguousarray(fw_hw_b, dtype=np.float32),
        "bw_b": np.ascontiguousarray(bw_hw_b, dtype=np.float32),
    }
    in_maps = []
    for c in range(CORES):
        shard = np.asarray(inputs[c * BL:(c + 1) * BL], dtype=np.float32)
        x_feat = np.ascontiguousarray(np.transpose(shard, (2, 0, 1)))  # [H, BL, S]
        in_maps.append({"x_t": x_feat, **common})
    return in_maps


def assemble_output(results):
    """Per-core out [L, 2, H, BL, S] -> full [L, B, S, 2H]."""
    full = np.empty((L, B, S, 2 * H), dtype=np.float32)
    for c, r in enumerate(results):
        o = r["out"]  # [L, 2, H, BL, S]
        # [L, 2, H, BL, S] -> [L, BL, S, 2, H] -> [L, BL, S, 2H]
        full[:, c * BL:(c + 1) * BL] = np.transpose(
            o, (0, 3, 4, 1, 2)).reshape(L, BL, S, 2 * H)
    return full


_NC_CACHE = None


def kernel(inputs, masks, fw_pad, bw_pad, fw_w, bw_w,
           fw_hw_W, fw_hw_b, bw_hw_W, bw_hw_b):
    del masks  # all-ones; unused by the reference computation
    global _NC_CACHE
    if _NC_CACHE is None:
        _NC_CACHE = build_nc()
    in_maps = prepare_in_maps(inputs, fw_pad, bw_pad, fw_w, bw_w,
                              fw_hw_W, fw_hw_b, bw_hw_W, bw_hw_b)
    res = bass_utils.run_bass_kernel_spmd(_NC_CACHE, in_maps,
                                          core_ids=list(range(CORES)))
    return assemble_output(res.results)


if __name__ == "__main__":
    nc = build_nc()
    print("built ok")
